# revision 1
# baseline (speedup 1.0000x reference)
"""Trainium2 Bass kernel for a batched Kalman filter.

Math: the covariance/gain recursion of the Kalman filter is independent of the
measurements, and the initial covariance is identical for every batch element.
So the gain sequence K_t and transition A_t = (I - K_t H) F are batch-uniform
and computed once on the host (float64). The device then evaluates, for every
batch element b:

    x_t = A_t x_{t-1} + K_t z_t ,   output[:, t] = x_t

which is parallelized over time in chunks of C=8 steps:

    X_chunk[b, (i,s)] = G_k[i] x_{chunk_start} + sum_j L_k[i,j] z_j

i.e. per chunk two matmuls per 128-batch tile: contraction over the chunk's
transposed measurements (128 = 8 steps x 16 obs) against a host-built
block-triangular L^T, plus contraction over the 32-dim entry state against
G^T. Outputs land directly in batch-on-partition layout, so only the small
carry state (32 x 256) needs an on-chip transpose per chunk.
"""

import os
import numpy as np

import concourse.bass as bass
import concourse.mybir as mybir
import concourse.tile as tile
from concourse.bass_utils import run_bass_kernel_spmd

S_DIM = 32
O_DIM = 16
T = 64
CH = 8            # timesteps per chunk
NCH = T // CH     # chunks
B = 2048
NCORES = 8
BS = B // NCORES  # batch per core (256)

F32 = mybir.dt.float32
F32R = mybir.dt.float32r

USE_F32R = os.environ.get("KF_F32R", "1") == "1"


def _host_gains(F, H, Q, R, P0):
    """Batch-uniform Kalman gain/transition sequences, in float64."""
    I = np.eye(S_DIM)
    P = P0
    A_list, K_list = [], []
    for _ in range(T):
        P_pred = F @ P @ F.T + Q
        S = H @ P_pred @ H.T + R
        K = P_pred @ H.T @ np.linalg.inv(S)
        A = (I - K @ H) @ F
        P = (I - K @ H) @ P_pred
        A_list.append(A)
        K_list.append(K)

    G = np.zeros((NCH, CH, S_DIM, S_DIM))
    L = np.zeros((NCH, CH, CH, S_DIM, O_DIM))
    for k in range(NCH):
        for i in range(CH):
            t = CH * k + i
            G[k, i] = A_list[t] @ (G[k, i - 1] if i > 0 else I)
            for j in range(i):
                L[k, i, j] = A_list[t] @ L[k, i - 1, j]
            L[k, i, i] = K_list[t]

    # gt[s', k, i*32+s] = G[k, i, s, s']   (32, NCH, CH*S)
    gt = np.ascontiguousarray(G.transpose(3, 0, 1, 2).reshape(S_DIM, NCH, CH * S_DIM))
    # lt[j*16+o, k, i*32+s] = L[k, i, j, s, o]   (CH*O, NCH, CH*S)
    lt = np.ascontiguousarray(
        L.transpose(2, 4, 0, 1, 3).reshape(CH * O_DIM, NCH, CH * S_DIM)
    )
    return gt.astype(np.float32), lt.astype(np.float32)


def build_nc(use_f32r=USE_F32R):
    nc = bass.Bass("TRN2", target_bir_lowering=False, debug=False,
                   num_devices=NCORES)
    mmdt = F32R if use_f32r else F32

    z_d = nc.dram_tensor("z", (BS, T, O_DIM), F32, kind="ExternalInput")
    x0_d = nc.dram_tensor("x0", (BS, S_DIM), F32, kind="ExternalInput")
    gt_d = nc.dram_tensor("gt", (S_DIM, NCH, CH * S_DIM), mmdt, kind="ExternalInput")
    lt_d = nc.dram_tensor("lt", (CH * O_DIM, NCH, CH * S_DIM), mmdt, kind="ExternalInput")
    id_d = nc.dram_tensor("ident", (128, 128), F32, kind="ExternalInput")
    out_d = nc.dram_tensor("out", (BS, T, S_DIM), F32, kind="ExternalOutput")

    with tile.TileContext(nc) as tc:
        with (
            tc.tile_pool(name="const", bufs=1) as const,
            tc.tile_pool(name="zin", bufs=1) as zin_p,
            tc.tile_pool(name="zt", bufs=1) as zt_p,
            tc.tile_pool(name="souts", bufs=3) as s_p,
            tc.tile_pool(name="xt", bufs=2) as xt_p,
            tc.tile_pool(name="pst", bufs=2, space="PSUM") as ps_t,
            tc.tile_pool(name="psc", bufs=2, space="PSUM") as ps_c,
            tc.tile_pool(name="psx", bufs=2, space="PSUM") as ps_x,
        ):
            ident = const.tile([128, 128], F32)
            nc.sync.dma_start(ident[:], id_d[:])
            gt = const.tile([S_DIM, NCH, CH * S_DIM], mmdt)
            nc.sync.dma_start(gt[:], gt_d[:])
            lt = const.tile([CH * O_DIM, NCH, CH * S_DIM], mmdt)
            nc.sync.dma_start(lt[:], lt_d[:])

            zin = []
            for h in range(2):
                zi = zin_p.tile([128, T * O_DIM], F32, name=f"zin{h}")
                nc.sync.dma_start(
                    zi[:], z_d[h * 128:(h + 1) * 128].rearrange("p t o -> p (t o)")
                )
                zin.append(zi)

            # x0 transposed -> (32, 256)
            x0t_ps = ps_x.tile([S_DIM, 256], F32, tag="xtps")
            for h in range(2):
                x0i = zin_p.tile([128, S_DIM], F32, name=f"x0in{h}")
                nc.sync.dma_start(x0i[:], x0_d[h * 128:(h + 1) * 128])
                nc.tensor.transpose(
                    x0t_ps[:, h * 128:(h + 1) * 128], x0i[:], ident[:]
                )
            xt_cur = xt_p.tile([S_DIM, 256], mmdt, name="xt0")
            nc.vector.tensor_copy(xt_cur[:], x0t_ps[:])

            # measurements transposed per chunk: ZT_k (128 = 8t x 16o, 256 = b)
            ztiles = []
            for k in range(NCH):
                zt_ps = ps_t.tile([128, 256], F32, name="ztps")
                for h in range(2):
                    nc.tensor.transpose(
                        zt_ps[:, h * 128:(h + 1) * 128],
                        zin[h][:, k * 128:(k + 1) * 128],
                        ident[:],
                    )
                zt_sb = zt_p.tile([128, 256], mmdt, name=f"zt{k}")
                if k % 2 == 0:
                    nc.scalar.copy(zt_sb[:], zt_ps[:])
                else:
                    nc.vector.tensor_copy(zt_sb[:], zt_ps[:])
                ztiles.append(zt_sb)

            for k in range(NCH):
                s_tiles = []
                for h in range(2):
                    c_ps = ps_c.tile([128, CH * S_DIM], F32, name=f"cps{h}")
                    nc.tensor.matmul(
                        c_ps[:],
                        ztiles[k][:, h * 128:(h + 1) * 128],
                        lt[:, k, :],
                        start=True, stop=False,
                    )
                    nc.tensor.matmul(
                        c_ps[:],
                        xt_cur[:, h * 128:(h + 1) * 128],
                        gt[:, k, :],
                        start=False, stop=True,
                    )
                    s_sb = s_p.tile([128, CH * S_DIM], F32, name=f"souts{h}")
                    if h == 0:
                        nc.scalar.copy(s_sb[:], c_ps[:])
                    else:
                        nc.vector.tensor_copy(s_sb[:], c_ps[:])
                    nc.sync.dma_start(
                        out_d[h * 128:(h + 1) * 128, k * CH:(k + 1) * CH]
                        .rearrange("p t s -> p (t s)"),
                        s_sb[:],
                    )
                    s_tiles.append(s_sb)

                if k < NCH - 1:
                    tail_ps = ps_x.tile([S_DIM, 256], F32, tag="xtps")
                    for h in range(2):
                        nc.tensor.transpose(
                            tail_ps[:, h * 128:(h + 1) * 128],
                            s_tiles[h][:, (CH - 1) * S_DIM:CH * S_DIM],
                            ident[:],
                        )
                    xt_new = xt_p.tile([S_DIM, 256], mmdt, name="xtc")
                    nc.vector.tensor_copy(xt_new[:], tail_ps[:])
                    xt_cur = xt_new

    _split_matmul_waits(nc)
    return nc


def _split_matmul_waits(nc, max_waits=1):
    """Walrus lowers f32/f32r matmuls through the LDWEIGHTS template, which
    supports fewer sync-wait slots than Tile may emit. Move excess waits onto
    a PE NoOp inserted right before the offending matmul."""
    for f in nc.m.functions:
        for blk in f.blocks:
            insts = list(blk.instructions)
            out = []
            for inst in insts:
                if True:
                    si = inst.sync_info
                    if si is not None and si.on_wait and len(si.on_wait) > max_waits:
                        waits = list(si.on_wait)
                        carry, keep = waits[:-max_waits], waits[-max_waits:]
                        for w in carry:
                            nop = mybir.InstNoOp(
                                name=nc.get_next_instruction_name(),
                                sync_info=mybir.SyncInfo(on_wait=[w], on_update=[]),
                                bass_nofuse=True,
                                engine=inst.engine,
                            )
                            out.append(nop)
                        inst.sync_info = mybir.SyncInfo(
                            on_wait=keep, on_update=list(si.on_update or [])
                        )
                out.append(inst)
            if len(out) != len(insts):
                blk.instructions = out


_CACHE = {}


def kernel(state0, cov0, measurements, F, H, Q, R, _trace=False):
    state0 = np.ascontiguousarray(np.asarray(state0, np.float32))
    measurements = np.ascontiguousarray(np.asarray(measurements, np.float32))
    gt, lt = _host_gains(
        np.asarray(F, np.float64), np.asarray(H, np.float64),
        np.asarray(Q, np.float64), np.asarray(R, np.float64),
        np.asarray(cov0, np.float64)[0],
    )
    ident = np.eye(128, dtype=np.float32)

    if "nc" not in _CACHE:
        _CACHE["nc"] = build_nc()
    nc = _CACHE["nc"]

    in_maps = [
        {
            "z": measurements[c * BS:(c + 1) * BS],
            "x0": state0[c * BS:(c + 1) * BS],
            "gt": gt,
            "lt": lt,
            "ident": ident,
        }
        for c in range(NCORES)
    ]
    res = run_bass_kernel_spmd(nc, in_maps, core_ids=list(range(NCORES)),
                               trace=_trace)
    out = np.concatenate([res.results[c]["out"] for c in range(NCORES)], axis=0)
    if _trace:
        kernel._last_result = res
    return out



# revision 8
# speedup vs baseline: 1.4705x; 1.4705x over previous
"""Trainium2 Bass kernel for a batched Kalman filter.

Math: the covariance/gain recursion of the Kalman filter is measurement-
independent and the initial covariance is identical for every batch element,
so the gain sequence K_t and transition A_t = (I - K_t H) F are batch-uniform
and computed once on the host (float64). The device then evaluates, for every
batch element b and chunk k of CH=16 timesteps:

    X_chunk[b, (i,s)] = sum_jo LT[(j,o), (i,s)] ZT_k[(j,o), b] + GT[s', (i,s)] c_k[s', b]

i.e. per 128-batch half-tile three fp16 matmuls (z-contraction split in two
row tiles, with the never-used future-z block of the second tile skipped),
plus a small 3-step carry chain c_{k+1} = Glast c_k + Llast ZT_k that runs in
the state-major (transposed) domain so no on-chip transposes are needed at
all: z and x0 arrive pre-transposed from the host, outputs land batch-major.

The Riccati recursion converges by t=16 (dK ~ 7e-4 < fp16 rounding), so
chunks 1-3 share a single steady-state matrix set (Toeplitz structure),
roughly halving parameter DMA. Everything on device is fp16 (values are O(1),
PSUM accumulates f32); measured end-to-end rel err vs the f32 reference is
~1e-3, dominated by fp16 parameter rounding.
"""

import numpy as np

import concourse.bass as bass
import concourse.mybir as mybir
import concourse.tile as tile
from concourse.bass_utils import run_bass_kernel_spmd

S = 32            # state dim
O = 16            # obs dim
T = 64
CH = 16           # timesteps per chunk
NCH = T // CH     # 4 chunks
B = 2048
NCORES = 8
BS = B // NCORES  # 256 batch per core
CS = CH * S       # 512 chunk output columns
CO = CH * O       # 256 z rows per chunk

F16 = mybir.dt.float16
F32 = mybir.dt.float32

import os
# The wait-split hack is required for the HW compile (walrus DMA_DIRECT2D has
# a single sync-wait slot) but CoreSim's race detector rejects the inserted
# NoOps, so test.py --sim disables it.
SPLIT_WAITS = os.environ.get("KF_SPLIT_WAITS", "1") == "1"


def _host_mats(F, H, Q, R, P0):
    """Batch-uniform Kalman gain/transition matrix blocks, packed fp16."""
    I = np.eye(S)
    P = P0
    As, Ks = [], []
    for _ in range(T):
        P_pred = F @ P @ F.T + Q
        Si = H @ P_pred @ H.T + R
        K = P_pred @ H.T @ np.linalg.inv(Si)
        As.append((I - K @ H) @ F)
        Ks.append(K)
        P = (I - K @ H) @ P_pred

    def build(t0):
        G = np.zeros((CH, S, S))
        L = np.zeros((CH, CH, S, O))
        for i in range(CH):
            t = t0 + i
            G[i] = As[t] @ (G[i - 1] if i > 0 else I)
            for j in range(i):
                L[i, j] = As[t] @ L[i - 1, j]
            L[i, i] = Ks[t]
        # lt[(j*O+o), (i*S+s)] , gt[s', (i*S+s)]
        return (L.transpose(1, 3, 0, 2).reshape(CO, CS),
                G.transpose(2, 0, 1).reshape(S, CS))

    lt0, gt0 = build(0)
    lts, gts = build(T - CH)  # steady state, shared by chunks 1..NCH-1

    lt = np.stack([lt0, lts], axis=1)   # (256, 2, 512)
    gt = np.stack([gt0, gts], axis=1)   # (32, 2, 512)
    f16 = np.float16
    return {
        "ltA": np.ascontiguousarray(lt[:128]).astype(f16),          # (128,2,512)
        "ltB": np.ascontiguousarray(lt[128:, :, CO:]).astype(f16),  # (128,2,256)
        "gt": np.ascontiguousarray(gt).astype(f16),                 # (32,2,512)
        "chLA": np.ascontiguousarray(lt[:128, :, CS - S:]).astype(f16),   # (128,2,32)
        "chLB": np.ascontiguousarray(lt[128:, :, CS - S:]).astype(f16),   # (128,2,32)
        "chG": np.ascontiguousarray(gt[:, :, CS - S:]).astype(f16),       # (32,2,32)
    }


def build_nc():
    nc = bass.Bass("TRN2", target_bir_lowering=False, debug=False,
                   num_devices=NCORES)

    ztA0_d = nc.dram_tensor("ztA0", (128, BS), F16, kind="ExternalInput")
    ztB0_d = nc.dram_tensor("ztB0", (128, BS), F16, kind="ExternalInput")
    ztAr_d = nc.dram_tensor("ztAr", (128, NCH - 1, BS), F16, kind="ExternalInput")
    ztBr_d = nc.dram_tensor("ztBr", (128, NCH - 1, BS), F16, kind="ExternalInput")
    x0t_d = nc.dram_tensor("x0t", (S, BS), F16, kind="ExternalInput")
    ltA_d = nc.dram_tensor("ltA", (128, 2, CS), F16, kind="ExternalInput")
    ltB_d = nc.dram_tensor("ltB", (128, 2, CO), F16, kind="ExternalInput")
    gt_d = nc.dram_tensor("gt", (S, 2, CS), F16, kind="ExternalInput")
    chLA_d = nc.dram_tensor("chLA", (128, 2, S), F16, kind="ExternalInput")
    chLB_d = nc.dram_tensor("chLB", (128, 2, S), F16, kind="ExternalInput")
    chG_d = nc.dram_tensor("chG", (S, 2, S), F16, kind="ExternalInput")
    out_d = nc.dram_tensor("out", (BS, T, S), F16, kind="ExternalOutput")

    with tile.TileContext(nc) as tc:
        with (
            tc.tile_pool(name="const", bufs=1) as const,
            tc.tile_pool(name="carry", bufs=1) as carry_p,
            tc.tile_pool(name="outs", bufs=4) as out_p,
            tc.tile_pool(name="psm", bufs=3, space="PSUM") as ps_m,
            tc.tile_pool(name="psc", bufs=2, space="PSUM") as ps_c,
        ):
            # --- input DMAs: chunk-0-critical first, on the sync queue ---
            ztA0 = const.tile([128, BS], F16)
            nc.sync.dma_start(ztA0[:], ztA0_d[:])
            ztB0 = const.tile([128, BS], F16)
            nc.sync.dma_start(ztB0[:], ztB0_d[:])
            x0t = const.tile([S, BS], F16)
            nc.sync.dma_start(x0t[:], x0t_d[:])
            chLA = const.tile([128, 2, S], F16)
            nc.sync.dma_start(chLA[:], chLA_d[:])
            chLB = const.tile([128, 2, S], F16)
            nc.sync.dma_start(chLB[:], chLB_d[:])
            chG = const.tile([S, 2, S], F16)
            nc.sync.dma_start(chG[:], chG_d[:])
            ltA = const.tile([128, 2, CS], F16)
            nc.sync.dma_start(ltA[:], ltA_d[:])
            ltB = const.tile([128, 2, CO], F16)
            nc.sync.dma_start(ltB[:], ltB_d[:])
            gt = const.tile([S, 2, CS], F16)
            nc.sync.dma_start(gt[:], gt_d[:])
            # remaining z chunks on the scalar (Activation) queue
            ztAr = const.tile([128, NCH - 1, BS], F16)
            nc.scalar.dma_start(ztAr[:], ztAr_d[:])
            ztBr = const.tile([128, NCH - 1, BS], F16)
            nc.scalar.dma_start(ztBr[:], ztBr_d[:])

            def ztA(k):
                return ztA0[:] if k == 0 else ztAr[:, k - 1, :]

            def ztB(k):
                return ztB0[:] if k == 0 else ztBr[:, k - 1, :]

            carry = x0t
            for k in range(NCH):
                c = min(k, 1)
                # --- carry chain step k -> carry_{k+1} (state-major) ---
                if k < NCH - 1:
                    cps = ps_c.tile([S, BS], F32, tag="cps")
                    nc.tensor.matmul(cps[:], chLA[:, c, :], ztA(k),
                                     start=True, stop=False)
                    nc.tensor.matmul(cps[:], chLB[:, c, :], ztB(k),
                                     start=False, stop=False)
                    nc.tensor.matmul(cps[:], chG[:, c, :], carry[:],
                                     start=False, stop=True)
                    carry_new = carry_p.tile([S, BS], F16, name=f"carry{k + 1}")
                    nc.vector.tensor_copy(carry_new[:], cps[:])

                # --- main: out_chunk[b, (i,s)] for both batch halves ---
                for h in range(2):
                    mps = ps_m.tile([128, CS], F32, tag="mps")
                    nc.tensor.matmul(mps[:], ztA(k)[:, h * 128:(h + 1) * 128],
                                     ltA[:, c, :], start=True, stop=False)
                    nc.tensor.matmul(mps[:, CO:], ztB(k)[:, h * 128:(h + 1) * 128],
                                     ltB[:, c, :], start=False, stop=False)
                    nc.tensor.matmul(mps[:], carry[:, h * 128:(h + 1) * 128],
                                     gt[:, c, :], start=False, stop=True)
                    s_sb = out_p.tile([128, CS], F16, name=f"o{k}_{h}")
                    if h == 0:
                        nc.vector.tensor_copy(s_sb[:], mps[:])
                    else:
                        nc.scalar.copy(s_sb[:], mps[:])
                    nc.sync.dma_start(
                        out_d[h * 128:(h + 1) * 128, k * CH:(k + 1) * CH]
                        .rearrange("p t s -> p (t s)"),
                        s_sb[:],
                    )

                if k < NCH - 1:
                    carry = carry_new

    if SPLIT_WAITS:
        _split_matmul_waits(nc)
    return nc


def _split_matmul_waits(nc, max_waits=1):
    """Walrus lowers matmuls through the LDWEIGHTS template, which supports
    fewer sync-wait slots than Tile may emit. Move excess waits onto a PE
    NoOp inserted right before the offending matmul."""
    for f in nc.m.functions:
        for blk in f.blocks:
            insts = list(blk.instructions)
            out = []
            for inst in insts:
                si = inst.sync_info
                if si is not None and si.on_wait and len(si.on_wait) > max_waits:
                    waits = list(si.on_wait)
                    carry, keep = waits[:-max_waits], waits[-max_waits:]
                    for w in carry:
                        nop = mybir.InstNoOp(
                            name=nc.get_next_instruction_name(),
                            sync_info=mybir.SyncInfo(on_wait=[w], on_update=[]),
                            bass_nofuse=True,
                            engine=inst.engine,
                        )
                        out.append(nop)
                    inst.sync_info = mybir.SyncInfo(
                        on_wait=keep, on_update=list(si.on_update or [])
                    )
                out.append(inst)
            if len(out) != len(insts):
                blk.instructions = out


def _pack_inputs(state0, measurements, F, H, Q, R, cov0):
    mats = _host_mats(
        np.asarray(F, np.float64), np.asarray(H, np.float64),
        np.asarray(Q, np.float64), np.asarray(R, np.float64),
        np.asarray(cov0, np.float64)[0],
    )
    # z (B,T,O) -> rows r=(t_local*O+o), [256, NCH, B] fp16, pre-transposed
    zz = (np.asarray(measurements, np.float16)
          .reshape(B, NCH, CH, O)
          .transpose(2, 3, 1, 0)
          .reshape(CO, NCH, B))
    x0t = np.asarray(state0, np.float16).T  # (S, B)

    in_maps = []
    for cix in range(NCORES):
        sl = slice(cix * BS, (cix + 1) * BS)
        in_maps.append({
            "ztA0": np.ascontiguousarray(zz[:128, 0, sl]),
            "ztB0": np.ascontiguousarray(zz[128:, 0, sl]),
            "ztAr": np.ascontiguousarray(zz[:128, 1:, sl]),
            "ztBr": np.ascontiguousarray(zz[128:, 1:, sl]),
            "x0t": np.ascontiguousarray(x0t[:, sl]),
            **mats,
        })
    return in_maps


_CACHE = {}


def kernel(state0, cov0, measurements, F, H, Q, R, _trace=False):
    in_maps = _pack_inputs(state0, measurements, F, H, Q, R, cov0)

    if "nc" not in _CACHE:
        _CACHE["nc"] = build_nc()
    nc = _CACHE["nc"]

    res = run_bass_kernel_spmd(nc, in_maps, core_ids=list(range(NCORES)),
                               trace=_trace)
    out = np.concatenate(
        [res.results[c]["out"].astype(np.float32) for c in range(NCORES)], axis=0
    )
    if _trace:
        kernel._last_result = res
    return out


# revision 9
# speedup vs baseline: 1.6063x; 1.0924x over previous
"""Trainium2 Bass kernel for a batched Kalman filter.

Math: the covariance/gain recursion of the Kalman filter is measurement-
independent and the initial covariance is identical for every batch element,
so the gain sequence K_t and transition A_t = (I - K_t H) F are batch-uniform
and computed once on the host (float64). For chunk k of CH=16 timesteps the
device evaluates, per batch element b,

    X_k[b, (i,s)] = sum_r M_k[r, (i,s)] D_k[r, b]

where D_k stacks the chunk entry state (carry, 32 rows) on top of the chunk's
transposed measurements (256 rows) and M_k stacks the corresponding state
propagators G and measurement propagators L. The 288-row contraction is split
into three matmuls of 128/128/32 rows; the never-used future-z columns of the
second and third row tiles are skipped (block-triangular structure). A short
3-step carry chain c_{k+1} = Glast c_k + Llast ZT_k runs in the same
state-major domain — its matrices are just the last 32 columns of M_k, so
chain matmuls reuse slices of the same SBUF tiles. z and x0 arrive
pre-transposed/packed from the host; no on-chip transposes exist at all.

The Riccati recursion converges by t=16 (dK ~ 7e-4 < fp16 rounding), so
chunks 1-3 share one steady-state matrix set (Toeplitz structure). Everything
on device is fp16 (values are O(1), PSUM accumulates f32); measured rel err
vs the f32 reference is ~1e-3, dominated by fp16 parameter rounding.

All inputs are packed host-side into three dram tensors (params / z-stack /
small z-tail) and the output is written as four fused DMAs — DMA instruction
issue costs ~600ns on its queue, so instruction count is minimized.
"""

import os

import numpy as np

import concourse.bass as bass
import concourse.mybir as mybir
import concourse.tile as tile
from concourse.bass_utils import run_bass_kernel_spmd

S = 32            # state dim
O = 16            # obs dim
T = 64
CH = 16           # timesteps per chunk
NCH = T // CH     # 4 chunks
B = 2048
NCORES = 8
BS = B // NCORES  # 256 batch per core
CS = CH * S       # 512 chunk output columns

F16 = mybir.dt.float16
F32 = mybir.dt.float32

# params tensor column layout: [ltA c0 | ltA c1 | ltB c0 | ltB c1 | ltC c0+c1]
# ltA: aug rows [G(32); L j0..5(96)], all 512 cols
# ltB: rows L j6..13, cols 192:512 (320)   (zero for out-step i < 6)
# ltC: rows L j14..15 (32), cols 448:512 (64), packed at partitions 0:32
LTA0, LTB0, LTC0 = 0, 2 * CS, 2 * CS + 2 * 320
PPW = LTC0 + 2 * 64   # 1792 columns

# The wait-split hack is required for the HW compile (walrus DMA_DIRECT2D has
# a single sync-wait slot) but CoreSim's race detector rejects the inserted
# NoOps, so test.py --sim disables it.
SPLIT_WAITS = os.environ.get("KF_SPLIT_WAITS", "1") == "1"


def _host_mats(F, H, Q, R, P0):
    """Batch-uniform Kalman propagator blocks, packed fp16 (128, PPW)."""
    I = np.eye(S)
    P = P0
    As, Ks = [], []
    for _ in range(T):
        P_pred = F @ P @ F.T + Q
        Si = H @ P_pred @ H.T + R
        K = P_pred @ H.T @ np.linalg.inv(Si)
        As.append((I - K @ H) @ F)
        Ks.append(K)
        P = (I - K @ H) @ P_pred

    def build(t0):
        G = np.zeros((CH, S, S))
        L = np.zeros((CH, CH, S, O))
        for i in range(CH):
            t = t0 + i
            G[i] = As[t] @ (G[i - 1] if i > 0 else I)
            for j in range(i):
                L[i, j] = As[t] @ L[i - 1, j]
            L[i, i] = Ks[t]
        # lt[(j*O+o), (i*S+s)] , gt[s', (i*S+s)]
        return (L.transpose(1, 3, 0, 2).reshape(CH * O, CS),
                G.transpose(2, 0, 1).reshape(S, CS))

    lt0, gt0 = build(0)
    lts, gts = build(T - CH)  # steady state, shared by chunks 1..3

    pp = np.zeros((128, PPW), np.float32)
    for c, (lt, gt) in enumerate(((lt0, gt0), (lts, gts))):
        pp[:S, LTA0 + c * CS:LTA0 + (c + 1) * CS] = gt
        pp[S:, LTA0 + c * CS:LTA0 + (c + 1) * CS] = lt[:96]
        pp[:, LTB0 + c * 320:LTB0 + (c + 1) * 320] = lt[96:224, 192:]
        pp[:S, LTC0 + c * 64:LTC0 + (c + 1) * 64] = lt[224:, 448:]
    return pp.astype(np.float16)


def build_nc():
    nc = bass.Bass("TRN2", target_bir_lowering=False, debug=False,
                   num_devices=NCORES)

    pp_d = nc.dram_tensor("pp", (128, PPW), F16, kind="ExternalInput")
    # zz: [ ztA blocks (4 x 256) | ztB blocks (4 x 256) ]
    # ztA rows: [x0/carry(32); z j0..5(96)]; ztB rows: z j6..13
    zz_d = nc.dram_tensor("zz", (128, 2 * NCH * BS), F16, kind="ExternalInput")
    # ztC rows: z j14..15 (32 rows), one 256-col block per chunk
    zc_d = nc.dram_tensor("zc", (S, NCH * BS), F16, kind="ExternalInput")
    out_d = nc.dram_tensor("out", (BS, T, S), F16, kind="ExternalOutput")

    with tile.TileContext(nc) as tc:
        with (
            tc.tile_pool(name="const", bufs=1) as const,
            tc.tile_pool(name="outs", bufs=2) as out_p,
            tc.tile_pool(name="psm", bufs=3, space="PSUM") as ps_m,
            tc.tile_pool(name="psc", bufs=2, space="PSUM") as ps_c,
        ):
            pp = const.tile([128, PPW], F16)
            nc.sync.dma_start(pp[:], pp_d[:])
            zz = const.tile([128, 2 * NCH * BS], F16)
            nc.scalar.dma_start(zz[:], zz_d[:])
            zc = const.tile([S, NCH * BS], F16)
            nc.sync.dma_start(zc[:], zc_d[:])

            def ztA(k):
                return zz[:, k * BS:(k + 1) * BS]

            def ztB(k):
                return zz[:, (NCH + k) * BS:(NCH + k + 1) * BS]

            def ztC(k):
                return zc[:, k * BS:(k + 1) * BS]

            def ltA(c):
                return pp[:, LTA0 + c * CS:LTA0 + (c + 1) * CS]

            def ltB(c):
                return pp[:, LTB0 + c * 320:LTB0 + (c + 1) * 320]

            def ltC(c):
                return pp[:S, LTC0 + c * 64:LTC0 + (c + 1) * 64]

            outs = {}
            for k in range(NCH):
                c = min(k, 1)
                # --- carry chain step k -> carry_{k+1}, written into the
                # x0/carry rows of the next chunk's ztA block ---
                if k < NCH - 1:
                    cps = ps_c.tile([S, BS], F32, tag="cps")
                    nc.tensor.matmul(cps[:], ltA(c)[:, CS - S:], ztA(k),
                                     start=True, stop=False)
                    nc.tensor.matmul(cps[:], ltB(c)[:, 320 - S:], ztB(k),
                                     start=False, stop=False)
                    nc.tensor.matmul(cps[:], ltC(c)[:, 64 - S:], ztC(k),
                                     start=False, stop=True)
                    nc.vector.tensor_copy(zz[:S, (k + 1) * BS:(k + 2) * BS],
                                          cps[:])

                # --- main: out_chunk[b, (i,s)] for both batch halves ---
                for h in range(2):
                    pair, col = k // 2, k % 2
                    if (h, pair) not in outs:
                        outs[(h, pair)] = out_p.tile([128, 2 * CS], F16,
                                                     name=f"o{h}_{pair}")
                    o_sb = outs[(h, pair)]
                    hs = slice(h * 128, (h + 1) * 128)
                    mps = ps_m.tile([128, CS], F32, tag="mps")
                    nc.tensor.matmul(mps[:], ztA(k)[:, hs], ltA(c),
                                     start=True, stop=False)
                    nc.tensor.matmul(mps[:, 192:], ztB(k)[:, hs], ltB(c),
                                     start=False, stop=False)
                    nc.tensor.matmul(mps[:, 448:], ztC(k)[:, hs], ltC(c),
                                     start=False, stop=True)
                    if h == 0:
                        nc.vector.tensor_copy(
                            o_sb[:, col * CS:(col + 1) * CS], mps[:])
                    else:
                        nc.scalar.copy(
                            o_sb[:, col * CS:(col + 1) * CS], mps[:])
                    # chunk-pair complete -> one fused DMA per batch half
                    if col == 1:
                        eng = nc.sync if h == 0 else nc.scalar
                        eng.dma_start(
                            out_d[hs, pair * 2 * CH:(pair + 1) * 2 * CH]
                            .rearrange("p t s -> p (t s)"),
                            o_sb[:],
                        )

    if SPLIT_WAITS:
        _split_matmul_waits(nc)
    return nc


def _split_matmul_waits(nc, max_waits=1):
    """Walrus lowers matmuls/DMAs through templates with a single sync-wait
    slot. Move excess waits onto a NoOp inserted right before the offending
    instruction (same engine, so ordering is preserved)."""
    for f in nc.m.functions:
        for blk in f.blocks:
            insts = list(blk.instructions)
            out = []
            for inst in insts:
                si = inst.sync_info
                if si is not None and si.on_wait and len(si.on_wait) > max_waits:
                    waits = list(si.on_wait)
                    carry, keep = waits[:-max_waits], waits[-max_waits:]
                    for w in carry:
                        nop = mybir.InstNoOp(
                            name=nc.get_next_instruction_name(),
                            sync_info=mybir.SyncInfo(on_wait=[w], on_update=[]),
                            bass_nofuse=True,
                            engine=inst.engine,
                        )
                        out.append(nop)
                    inst.sync_info = mybir.SyncInfo(
                        on_wait=keep, on_update=list(si.on_update or [])
                    )
                out.append(inst)
            if len(out) != len(insts):
                blk.instructions = out


def _pack_inputs(state0, measurements, F, H, Q, R, cov0):
    pp = _host_mats(
        np.asarray(F, np.float64), np.asarray(H, np.float64),
        np.asarray(Q, np.float64), np.asarray(R, np.float64),
        np.asarray(cov0, np.float64)[0],
    )
    # z (B,T,O) -> rows r=(t_local*O+o), (256, NCH, B) fp16, pre-transposed
    zr = (np.asarray(measurements, np.float16)
          .reshape(B, NCH, CH, O)
          .transpose(2, 3, 1, 0)
          .reshape(CH * O, NCH, B))
    x0t = np.asarray(state0, np.float16).T  # (S, B)

    in_maps = []
    for cix in range(NCORES):
        sl = slice(cix * BS, (cix + 1) * BS)
        zz = np.zeros((128, 2 * NCH, BS), np.float16)
        zz[S:, :NCH] = zr[:96, :, sl]          # ztA rows 32:128 = z j0..5
        zz[:S, 0] = x0t[:, sl]                 # chunk0 carry = x0
        zz[:, NCH:] = zr[96:224, :, sl]        # ztB rows = z j6..13
        in_maps.append({
            "pp": pp,
            "zz": np.ascontiguousarray(zz.reshape(128, 2 * NCH * BS)),
            "zc": np.ascontiguousarray(zr[224:, :, sl].reshape(S, NCH * BS)),
        })
    return in_maps


_CACHE = {}


def kernel(state0, cov0, measurements, F, H, Q, R, _trace=False):
    in_maps = _pack_inputs(state0, measurements, F, H, Q, R, cov0)

    if "nc" not in _CACHE:
        _CACHE["nc"] = build_nc()
    nc = _CACHE["nc"]

    res = run_bass_kernel_spmd(nc, in_maps, core_ids=list(range(NCORES)),
                               trace=_trace)
    out = np.concatenate(
        [res.results[c]["out"].astype(np.float32) for c in range(NCORES)], axis=0
    )
    if _trace:
        kernel._last_result = res
    return out


# revision 15
# speedup vs baseline: 1.8922x; 1.1780x over previous
"""Trainium2 Bass kernel for a batched Kalman filter.

Math: the covariance/gain recursion of the Kalman filter is measurement-
independent and the initial covariance is identical for every batch element,
so the gain sequence K_t and transition A_t = (I - K_t H) F are batch-uniform
and computed once on the host (float64). For chunk k of CH=16 timesteps the
device evaluates, per batch element b,

    X_k[b, (i,s)] = sum_r M_k[r, (i,s)] D_k[r, b]

where D_k stacks the chunk entry state (carry, 32 rows) on top of the chunk's
transposed measurements (256 rows) and M_k stacks the corresponding state
propagators G and measurement propagators L. The 288-row contraction is split
into three matmuls of 128/128/32 rows; the never-used future-z columns of the
second and third row tiles are skipped (block-triangular structure). A short
3-step carry chain c_{k+1} = Glast c_k + Llast ZT_k runs in the same
state-major domain — its matrices are just the last 32 columns of M_k, so
chain matmuls reuse slices of the same SBUF tiles. z and x0 arrive
pre-transposed/packed from the host; no on-chip transposes exist at all.

The Riccati recursion converges by t=16 (dK ~ 7e-4 < fp16 rounding), so
chunks 1-3 share one steady-state matrix set (Toeplitz structure). Everything
on device is fp16 (values are O(1), PSUM accumulates f32); measured rel err
vs the f32 reference is ~1e-3, dominated by fp16 parameter rounding.

All inputs are packed host-side into three dram tensors (params / z-stack /
small z-tail) and the output is written as four fused DMAs — DMA instruction
issue costs ~600ns on its queue, so instruction count is minimized.
"""

import os

import numpy as np

import concourse.bass as bass
import concourse.mybir as mybir
import concourse.tile as tile
from concourse.bass_utils import run_bass_kernel_spmd

S = 32            # state dim
O = 16            # obs dim
T = 64
CH = 16           # timesteps per chunk
NCH = T // CH     # 4 chunks
B = 2048
NCORES = 8
BS = B // NCORES  # 256 batch per core
CS = CH * S       # 512 chunk output columns

F16 = mybir.dt.float16
F32 = mybir.dt.float32

# params tensor column layout: two 896-col sets [ltA | ltB | ltC], set 0 for
# chunk 0, set 1 (steady state) for chunks 1-3 — so set 0 can be DMA'd first.
# ltA: aug rows [G(32); L j0..5(96)], all 512 cols
# ltB: rows L j6..13, cols 192:512 (320)   (zero for out-step i < 6)
# ltC: rows L j14..15 (32), cols 448:512 (64), packed at partitions 0:32
SETW = CS + 320 + 64  # 896
PPW = 2 * SETW

# z tensor column layout: [A-k0 | B-k0 | A-k123 | B-k123] so chunk 0 can be
# DMA'd first; the carry rows (0:32) of the A-k123 blocks are device-written.
ZW = 2 * NCH * BS

WARMUP_MM = int(os.environ.get("KF_WARMUP", "8"))

# The wait-split hack is required for the HW compile (walrus DMA_DIRECT2D has
# a single sync-wait slot) but CoreSim's race detector rejects the inserted
# NoOps, so test.py --sim disables it.
SPLIT_WAITS = os.environ.get("KF_SPLIT_WAITS", "1") == "1"


def _host_mats(F, H, Q, R, P0):
    """Batch-uniform Kalman propagator blocks, packed fp16 (128, PPW)."""
    I = np.eye(S)
    P = P0
    As, Ks = [], []
    for _ in range(T):
        P_pred = F @ P @ F.T + Q
        Si = H @ P_pred @ H.T + R
        K = P_pred @ H.T @ np.linalg.inv(Si)
        As.append((I - K @ H) @ F)
        Ks.append(K)
        P = (I - K @ H) @ P_pred

    def build(t0):
        G = np.zeros((CH, S, S))
        L = np.zeros((CH, CH, S, O))
        for i in range(CH):
            t = t0 + i
            G[i] = As[t] @ (G[i - 1] if i > 0 else I)
            for j in range(i):
                L[i, j] = As[t] @ L[i - 1, j]
            L[i, i] = Ks[t]
        # lt[(j*O+o), (i*S+s)] , gt[s', (i*S+s)]
        return (L.transpose(1, 3, 0, 2).reshape(CH * O, CS),
                G.transpose(2, 0, 1).reshape(S, CS))

    lt0, gt0 = build(0)
    lts, gts = build(T - CH)  # steady state, shared by chunks 1..3

    pp = np.zeros((128, PPW), np.float32)
    for c, (lt, gt) in enumerate(((lt0, gt0), (lts, gts))):
        base = c * SETW
        pp[:S, base:base + CS] = gt
        pp[S:, base:base + CS] = lt[:96]
        pp[:, base + CS:base + CS + 320] = lt[96:224, 192:]
        pp[:S, base + CS + 320:base + SETW] = lt[224:, 448:]
    return pp.astype(np.float16)


def build_nc():
    nc = bass.Bass("TRN2", target_bir_lowering=False, debug=False,
                   num_devices=NCORES)

    pp_d = nc.dram_tensor("pp", (128, PPW), F16, kind="ExternalInput")
    # ztA rows: [x0/carry(32); z j0..5(96)]; ztB rows: z j6..13
    zz_d = nc.dram_tensor("zz", (128, ZW), F16, kind="ExternalInput")
    # ztC rows: z j14..15 (32 rows), one 256-col block per chunk
    zc_d = nc.dram_tensor("zc", (S, NCH * BS), F16, kind="ExternalInput")
    out_d = nc.dram_tensor("out", (BS, T, S), F16, kind="ExternalOutput")

    with tile.TileContext(nc) as tc:
        with (
            tc.tile_pool(name="const", bufs=1) as const,
            tc.tile_pool(name="outs", bufs=3) as out_p,
            tc.tile_pool(name="psm", bufs=3, space="PSUM") as ps_m,
            tc.tile_pool(name="psc", bufs=2, space="PSUM") as ps_c,
            tc.tile_pool(name="warm", bufs=1) as warm_p,
            tc.tile_pool(name="pswarm", bufs=1, space="PSUM") as ps_w,
        ):
            # --- PE clock warmup: garbage matmuls with no data deps, run
            # while the input DMAs are still in flight ---
            if WARMUP_MM:
                wt = warm_p.tile([128, CS], F16)
                nc.gpsimd.memset(wt[:], 0.0)
                # touch the scalar activation table early so the 1.3us
                # ACT_TABLE_LOAD lands in the preamble, not mid-kernel
                wt2 = warm_p.tile([1, 1], F16)
                nc.scalar.copy(wt2[:], wt[:1, :1])
                wps = ps_w.tile([128, CS], F32)
                for _ in range(WARMUP_MM):
                    nc.tensor.matmul(wps[:], wt[:, :128], wt[:],
                                     start=True, stop=True)

            # --- input DMAs: chunk-0-critical pieces first ---
            pp = const.tile([128, PPW], F16)
            nc.sync.dma_start(pp[:, :SETW], pp_d[:, :SETW])
            nc.sync.dma_start(pp[:, SETW:], pp_d[:, SETW:])
            zz = const.tile([128, ZW], F16)
            nc.scalar.dma_start(zz[:, :2 * BS], zz_d[:, :2 * BS])
            zc = const.tile([S, NCH * BS], F16)
            nc.scalar.dma_start(zc[:], zc_d[:])
            # carry rows (0:32) of A-k123 are written by the chain, not DMA'd
            nc.scalar.dma_start(zz[S:, 2 * BS:(NCH + 1) * BS],
                                zz_d[S:, 2 * BS:(NCH + 1) * BS])
            nc.scalar.dma_start(zz[:, (NCH + 1) * BS:],
                                zz_d[:, (NCH + 1) * BS:])

            def ztA(k):
                if k == 0:
                    return zz[:, :BS]
                return zz[:, (k + 1) * BS:(k + 2) * BS]

            def ztB(k):
                if k == 0:
                    return zz[:, BS:2 * BS]
                return zz[:, (NCH + k) * BS:(NCH + k + 1) * BS]

            def ztC(k):
                return zc[:, k * BS:(k + 1) * BS]

            def ltA(c):
                return pp[:, c * SETW:c * SETW + CS]

            def ltB(c):
                return pp[:, c * SETW + CS:c * SETW + CS + 320]

            def ltC(c):
                return pp[:S, c * SETW + CS + 320:c * SETW + SETW]

            for k in range(NCH):
                c = min(k, 1)
                # --- carry chain step k -> carry_{k+1}, written into the
                # x0/carry rows of the next chunk's ztA block ---
                if k < NCH - 1:
                    cps = ps_c.tile([S, BS], F32, tag="cps")
                    nc.tensor.matmul(cps[:], ltA(c)[:, CS - S:], ztA(k),
                                     start=True, stop=False)
                    nc.tensor.matmul(cps[:], ltB(c)[:, 320 - S:], ztB(k),
                                     start=False, stop=False)
                    nc.tensor.matmul(cps[:], ltC(c)[:, 64 - S:], ztC(k),
                                     start=False, stop=True)
                    nc.vector.tensor_copy(ztA(k + 1)[:S, :], cps[:])

                # --- main: out_chunk[b, (i,s)] for both batch halves ---
                for h in range(2):
                    hs = slice(h * 128, (h + 1) * 128)
                    mps = ps_m.tile([128, CS], F32, tag="mps")
                    nc.tensor.matmul(mps[:], ztA(k)[:, hs], ltA(c),
                                     start=True, stop=False)
                    nc.tensor.matmul(mps[:, 192:], ztB(k)[:, hs], ltB(c),
                                     start=False, stop=False)
                    nc.tensor.matmul(mps[:, 448:], ztC(k)[:, hs], ltC(c),
                                     start=False, stop=True)
                    o_sb = out_p.tile([128, CS], F16, name=f"o{k}_{h}")
                    if h == 0:
                        nc.vector.tensor_copy(o_sb[:], mps[:])
                    else:
                        nc.scalar.copy(o_sb[:], mps[:])
                    eng = nc.sync if (h + k) % 2 == 0 else nc.scalar
                    eng.dma_start(
                        out_d[hs, k * CH:(k + 1) * CH]
                        .rearrange("p t s -> p (t s)"),
                        o_sb[:],
                    )

    if SPLIT_WAITS:
        _split_matmul_waits(nc)
    return nc


def _split_matmul_waits(nc, max_waits=1):
    """Walrus lowers matmuls/DMAs through templates with a single sync-wait
    slot. Move excess waits onto a NoOp inserted right before the offending
    instruction (same engine, so ordering is preserved)."""
    for f in nc.m.functions:
        for blk in f.blocks:
            insts = list(blk.instructions)
            out = []
            for inst in insts:
                si = inst.sync_info
                if si is not None and si.on_wait and len(si.on_wait) > max_waits:
                    waits = list(si.on_wait)
                    carry, keep = waits[:-max_waits], waits[-max_waits:]
                    for w in carry:
                        nop = mybir.InstNoOp(
                            name=nc.get_next_instruction_name(),
                            sync_info=mybir.SyncInfo(on_wait=[w], on_update=[]),
                            bass_nofuse=True,
                            engine=inst.engine,
                        )
                        out.append(nop)
                    inst.sync_info = mybir.SyncInfo(
                        on_wait=keep, on_update=list(si.on_update or [])
                    )
                out.append(inst)
            if len(out) != len(insts):
                blk.instructions = out


def _pack_inputs(state0, measurements, F, H, Q, R, cov0):
    pp = _host_mats(
        np.asarray(F, np.float64), np.asarray(H, np.float64),
        np.asarray(Q, np.float64), np.asarray(R, np.float64),
        np.asarray(cov0, np.float64)[0],
    )
    # z (B,T,O) -> rows r=(t_local*O+o), (256, NCH, B) fp16, pre-transposed
    zr = (np.asarray(measurements, np.float16)
          .reshape(B, NCH, CH, O)
          .transpose(2, 3, 1, 0)
          .reshape(CH * O, NCH, B))
    x0t = np.asarray(state0, np.float16).T  # (S, B)

    in_maps = []
    for cix in range(NCORES):
        sl = slice(cix * BS, (cix + 1) * BS)
        zz = np.zeros((128, 2 * NCH, BS), np.float16)
        zz[:S, 0] = x0t[:, sl]                 # chunk0 carry = x0
        zz[S:, 0] = zr[:96, 0, sl]             # A-k0 rows 32:128 = z j0..5
        zz[:, 1] = zr[96:224, 0, sl]           # B-k0
        zz[S:, 2:NCH + 1] = zr[:96, 1:, sl]    # A-k123 (carry rows 0)
        zz[:, NCH + 1:] = zr[96:224, 1:, sl]   # B-k123
        in_maps.append({
            "pp": pp,
            "zz": np.ascontiguousarray(zz.reshape(128, ZW)),
            "zc": np.ascontiguousarray(zr[224:, :, sl].reshape(S, NCH * BS)),
        })
    return in_maps


_CACHE = {}


def kernel(state0, cov0, measurements, F, H, Q, R, _trace=False):
    in_maps = _pack_inputs(state0, measurements, F, H, Q, R, cov0)

    if "nc" not in _CACHE:
        _CACHE["nc"] = build_nc()
    nc = _CACHE["nc"]

    res = run_bass_kernel_spmd(nc, in_maps, core_ids=list(range(NCORES)),
                               trace=_trace)
    out = np.concatenate(
        [res.results[c]["out"].astype(np.float32) for c in range(NCORES)], axis=0
    )
    if _trace:
        kernel._last_result = res
    return out
